# revision 1
# baseline (speedup 1.0000x reference)
"""Dilated LSTM (B=8, T=2048, C=1024, H=1024, D=4) on 8 trn2 NeuronCores.

Strategy: data-parallel over batch (core c <- batch item c, its 4 dilation
chains). Everything is core-local (no cross-core traffic; remote-DMA and
collectives are not usable in this deployment).

Per core:
  Phase A (GEMM): xg[t, g] = x[t, :] @ W_ih^T + (b_ih + b_hh), computed as
    9 K-tiles of 128 (the 9th K-tile is a host-appended ones-row carrying the
    bias), staged to DRAM (32 MB).
  Phase B (recurrence): 512 steps. Per step l, gates for the 4 chains:
    G[4, 4096] = hT_pack-stationary x W_hh^T-moving matmuls (8 K-tiles x 8
    PSUM chunks of 512), then DVE adds xg, ACT sigmoid/tanh, DVE/ACT c/h
    update, PE transposes h[4,1024] -> hT_pack[128, 8x4] for the next step.

Host reassembles y[b, t, h] from per-core y slabs.
"""

import sys

sys.path.insert(0, "/opt/trn_rl_repo")

import numpy as np

B, T, C, H, D = 8, 2048, 1024, 1024, 4
L = T // D  # 512 steps
G4 = 4 * H  # 4096 gates
KT = C // 128  # 8 k-tiles for x / h
NCORES = 8

_cached = {}

import os

N_STEPS = int(os.environ.get("DLSTM_STEPS", str(L)))  # dev override only
NO_DMA = os.environ.get("DLSTM_NODMA", "0") == "1"  # timing experiment only
PE_WARM = int(os.environ.get("DLSTM_WARM", "0"))  # filler MMs to hold PE clock


def _build(w_dtype_str):
    import concourse.bass as bass
    import concourse.bacc as bacc
    import concourse.mybir as mybir

    F32 = mybir.dt.float32
    R32 = mybir.dt.float32r
    USE_R32 = w_dtype_str == "f32r"
    if w_dtype_str == "bf16":
        WDT = mybir.dt.bfloat16
    elif USE_R32:
        WDT = R32  # split-fp32: full-rate matmul, near-fp32 precision
    else:
        WDT = F32
    XDT = R32 if USE_R32 else F32

    def mmcast(ap):
        return ap

    nc = bacc.Bacc(None, target_bir_lowering=False)

    # ---- I/O ----
    xT = nc.dram_tensor("xT", [C + 128, T], F32, kind="ExternalInput")
    wihT = nc.dram_tensor("wihT", [C + 128, G4], F32, kind="ExternalInput")
    whhT = nc.dram_tensor("whhT", [C, G4], F32 if USE_R32 else WDT, kind="ExternalInput")
    ident = nc.dram_tensor("ident", [4, 4], F32, kind="ExternalInput")
    y = nc.dram_tensor("y", [T, H], F32, kind="ExternalOutput")
    xg = nc.dram_tensor("xg", [T, G4], F32)  # internal staging, 32MB

    KTX = KT + 1  # 9 k-tiles incl bias row block

    from contextlib import ExitStack

    with ExitStack() as es_top:
        sems = {}
        for nm in ("ld_sem gp_sem ev_sem xw0_sem xw1_sem wh_sem ms_sem ps_sem ds_sem "
                   "as_sem cs_sem hs_sem tp_sem es_sem xr_sem yw0_sem yw1_sem "
                   "c2s_sem igs_sem id_sem").split():
            sems[nm] = es_top.enter_context(nc.semaphore(nm))
        ld_sem, gp_sem, ev_sem, wh_sem, ms_sem, ps_sem, ds_sem = (
            sems["ld_sem"], sems["gp_sem"], sems["ev_sem"],
            sems["wh_sem"], sems["ms_sem"], sems["ps_sem"], sems["ds_sem"])
        as_sem, cs_sem, hs_sem, tp_sem, es_sem, xr_sem = (
            sems["as_sem"], sems["cs_sem"], sems["hs_sem"], sems["tp_sem"],
            sems["es_sem"], sems["xr_sem"])
        xw_sems = (sems["xw0_sem"], sems["xw1_sem"])
        c2s_sem, igs_sem, id_sem = sems["c2s_sem"], sems["igs_sem"], sems["id_sem"]
        yw_sems = (sems["yw0_sem"], sems["yw1_sem"])
        # ---------------- Phase A: input GEMM ----------------
        with ExitStack() as es_a:
            xT_sb = es_a.enter_context(nc.sbuf_tensor("xT_sb", [128, KTX * T], XDT))
            wih_sb = es_a.enter_context(nc.sbuf_tensor("wih_sb", [128, KTX * 2048], XDT))
            stage = es_a.enter_context(nc.sbuf_tensor("stage", [128, 2 * 512], F32))
            gps = es_a.enter_context(nc.psum_tensor("gps", [128, 2 * 512], F32))
            blk = es_a.enter_context(nc.Block())
            # xT_sb k-tile k at cols [k*T, (k+1)*T); wih_sb k-tile at [k*2048, ..)
            def _emit_loads(s):
                for k in range(KTX):
                    s.dma_start(
                        xT_sb[:, k * T : (k + 1) * T], xT[k * 128 : (k + 1) * 128, :]
                    ).then_inc(ld_sem, 16)
                # pass 0 wih halves
                for k in range(KTX):
                    s.dma_start(
                        wih_sb[:, k * 2048 : (k + 1) * 2048],
                        wihT[k * 128 : (k + 1) * 128, 0:2048],
                    ).then_inc(ld_sem, 16)
                # pass 1 loads gated on pass-0 compute done
                s.wait_ge(gp_sem, 64)
                for k in range(KTX):
                    s.dma_start(
                        wih_sb[:, k * 2048 : (k + 1) * 2048],
                        wihT[k * 128 : (k + 1) * 128, 2048:4096],
                    ).then_inc(ld_sem, 16)

            if USE_R32:
                # loads must CAST f32 -> f32r (rounding producer); SWDGE only
                @blk.gpsimd
                def _(g):
                    _emit_loads(g)
            else:
                @blk.sync
                def _(s):
                    _emit_loads(s)

            @blk.tensor
            def _(t):
                for p in range(2):
                    t.wait_ge(ld_sem, 16 * KTX * (2 + p))
                    for m in range(16):  # bt tiles of 128
                        for ns in range(4):  # 512-col chunks within the half
                            idx = p * 64 + m * 4 + ns
                            if idx >= 2:
                                t.wait_ge(ev_sem, idx - 1)
                            bank = idx % 2
                            for k in range(KTX):
                                mmg = t.matmul(
                                    gps[:, bank * 512 : (bank + 1) * 512],
                                    mmcast(xT_sb[:, k * T + m * 128 : k * T + (m + 1) * 128]),
                                    mmcast(wih_sb[:, k * 2048 + ns * 512 : k * 2048 + (ns + 1) * 512]),
                                    start=(k == 0),
                                    stop=(k == KTX - 1),
                                )
                                if k == KTX - 1:
                                    mmg.then_inc(gp_sem, 1)

            @blk.vector
            def _(v):
                for p in range(2):
                    for m in range(16):
                        for ns in range(4):
                            idx = p * 64 + m * 4 + ns
                            bank = idx % 2
                            v.wait_ge(gp_sem, idx + 1)
                            if idx >= 2:
                                v.wait_ge(xw_sems[idx % 2], 16 * (idx // 2))
                            v.tensor_copy(
                                stage[:, bank * 512 : (bank + 1) * 512],
                                gps[:, bank * 512 : (bank + 1) * 512],
                            ).then_inc(ev_sem, 1)

            @blk.scalar
            def _(s):
                for p in range(2):
                    for m in range(16):
                        for ns in range(4):
                            idx = p * 64 + m * 4 + ns
                            bank = idx % 2
                            s.wait_ge(ev_sem, idx + 1)
                            s.dma_start(
                                xg[m * 128 : (m + 1) * 128, p * 2048 + ns * 512 : p * 2048 + (ns + 1) * 512],
                                stage[:, bank * 512 : (bank + 1) * 512],
                            ).then_inc(xw_sems[idx % 2], 16)
                s.wait_ge(xw_sems[0], 16 * 64)
                s.wait_ge(xw_sems[1], 16 * 64)

        # ---------------- Phase B: recurrence ----------------
        # wait: the matmul group semaphore convention above double-emits; see
        # the tensor block - it emits exactly one matmul per (k), with
        # then_inc only on the last. (Python ternary keeps one instruction.)
        LS = N_STEPS
        with ExitStack() as es_b:
            whh_sb = es_b.enter_context(nc.sbuf_tensor("whh_sb", [128, KT * G4], WDT))
            hT_pack = es_b.enter_context(nc.sbuf_tensor("hT_pack", [128, KT * 4], WDT))
            id_sb = es_b.enter_context(nc.sbuf_tensor("id_sb", [4, 4], F32))
            zero_sb = es_b.enter_context(nc.sbuf_tensor("zero_sb", [128, 512], F32))
            g_sb = es_b.enter_context(nc.sbuf_tensor("g_sb", [4, G4], F32))
            xg_sb = es_b.enter_context(nc.sbuf_tensor("xg_sb", [4, G4], F32))
            c_sb = es_b.enter_context(nc.sbuf_tensor("c_sb", [4, H], F32))
            c2_sb = es_b.enter_context(nc.sbuf_tensor("c2_sb", [4, H], F32))
            t_sb = es_b.enter_context(nc.sbuf_tensor("t_sb", [4, H], F32))
            h_sb = es_b.enter_context(nc.sbuf_tensor("h_sb", [4, 2 * H], F32))
            G_ps = es_b.enter_context(nc.psum_tensor("G_ps", [4, 6 * 512], F32))
            TA_ps = es_b.enter_context(nc.psum_tensor("TA_ps", [128, 16], F32))
            TB_ps = es_b.enter_context(nc.psum_tensor("TB_ps", [128, 16], F32))
            blk = es_b.enter_context(nc.Block())
            SIG = mybir.ActivationFunctionType.Sigmoid
            TANH = mybir.ActivationFunctionType.Tanh

            def gbank(n):  # psum column base for gate chunk n (chunks 6,7 alias 0,1)
                return (n % 6) * 512

            def _emit_whh_loads(s):
                for k in range(KT):
                    s.dma_start(
                        whh_sb[:, k * G4 : (k + 1) * G4], whhT[k * 128 : (k + 1) * 128, :]
                    ).then_inc(wh_sem, 16)

            @blk.sync
            def _(s):
                if not USE_R32:
                    _emit_whh_loads(s)
                s.dma_start(id_sb[:, :], ident[:, :]).then_inc(id_sem, 16)
                for l in range(LS if not NO_DMA else 1):
                    if l >= 1:
                        # single xg buffer: refill after step l-1's adds consumed it
                        s.wait_ge(ds_sem, 8 * l)
                    s.dma_start(xg_sb[:, :], xg[4 * l : 4 * l + 4, :]).then_inc(
                        xr_sem, 16
                    )

            @blk.scalar
            def _(s):
                # y writeback on the scalar (HWDGE) queue + the ACT work
                for l in range(LS):
                    # ACT: A1 sigmoid(i,f), A2 tanh(g), A3 sigmoid(o), A4 tanh(c)
                    s.wait_ge(ds_sem, 8 * l + 4)
                    s.activation(g_sb[:, 0:2048], g_sb[:, 0:2048], SIG).then_inc(as_sem, 1)
                    s.wait_ge(ds_sem, 8 * l + 6)
                    s.activation(g_sb[:, 2048:3072], g_sb[:, 2048:3072], TANH).then_inc(as_sem, 1)
                    s.wait_ge(ds_sem, 8 * l + 8)
                    s.activation(g_sb[:, 3072:4096], g_sb[:, 3072:4096], SIG).then_inc(as_sem, 1)
                    s.wait_ge(cs_sem, l + 1)
                    s.activation(t_sb[:, :], c_sb[:, :], TANH).then_inc(as_sem, 1)
                    # y writeback of h(l)
                    if NO_DMA and l != LS - 1:
                        continue
                    s.wait_ge(hs_sem, l + 1)
                    s.dma_start(
                        y[4 * l : 4 * l + 4, :], h_sb[:, (l % 2) * H : (l % 2 + 1) * H]
                    ).then_inc(yw_sems[l % 2], 16)
                if not NO_DMA:
                    s.wait_ge(yw_sems[0], 16 * ((LS + 1) // 2))
                    s.wait_ge(yw_sems[1], 16 * (LS // 2))

            @blk.gpsimd
            def _(g):
                if USE_R32:
                    _emit_whh_loads(g)  # SWDGE cast f32 -> f32r
                g.memset(zero_sb[:, :], 0.0)
                g.memset(c_sb[:, :], 0.0).then_inc(ms_sem, 1)

            @blk.tensor
            def _(t):
                t.wait_ge(wh_sem, 16 * KT)
                t.wait_ge(id_sem, 16)
                t.wait_ge(ms_sem, 2)
                for l in range(LS):
                    if l >= 1:
                        t.wait_ge(es_sem, 2 * l)  # hT(l-1) fully evacuated
                    if l >= 1:
                        t.wait_ge(ds_sem, 8 * (l - 1) + 6)  # banks 0..5 free
                    for n in range(8):
                        if n >= 6:
                            t.wait_ge(ds_sem, 8 * l + (n - 6) + 1)
                        for k in range(KT):
                            mm = t.matmul(
                                G_ps[:, gbank(n) : gbank(n) + 512],
                                mmcast(hT_pack[:, k * 4 : k * 4 + 4]),
                                mmcast(whh_sb[:, k * G4 + n * 512 : k * G4 + (n + 1) * 512]),
                                start=(k == 0),
                                stop=(k == KT - 1),
                            )
                            if k == KT - 1:
                                mm.then_inc(ps_sem, 1)
                    # transposes of h(l), two groups of 4 into banks A/B
                    t.wait_ge(hs_sem, l + 1)
                    for kk in range(KT):
                        tp = TA_ps if kk < 4 else TB_ps
                        ins = t.transpose(
                            tp[:, (kk % 4) * 4 : (kk % 4) * 4 + 4],
                            h_sb[:, (l % 2) * H + kk * 128 : (l % 2) * H + (kk + 1) * 128],
                            id_sb[:, :],
                        )
                        if kk % 4 == 3:
                            ins.then_inc(tp_sem, 1)
                    for _w in range(PE_WARM):
                        # filler on already-consumed G_ps bank 2; keeps the PE
                        # clock ramped while DVE evacuates hT (no new deps)
                        t.matmul(
                            G_ps[:, 2 * 512 : 3 * 512],
                            zero_sb[:, 0:4],
                            zero_sb[:, 0:512],
                            start=True,
                            stop=True,
                        )

            @blk.vector
            def _(v):
                v.wait_ge(ms_sem, 1)
                v.tensor_copy(hT_pack[:, :], zero_sb[:, 0 : KT * 4]).then_inc(ms_sem, 1)
                for l in range(LS):
                    # gate adds: g = G_ps + xg
                    for n in range(8):
                        v.wait_ge(ps_sem, 8 * l + n + 1)
                        if n == 0:
                            v.wait_ge(xr_sem, 16 * ((l + 1) if not NO_DMA else 1))
                        v.tensor_add(
                            g_sb[:, n * 512 : (n + 1) * 512],
                            G_ps[:, gbank(n) : gbank(n) + 512],
                            xg_sb[:, n * 512 : (n + 1) * 512],
                        ).then_inc(ds_sem, 1)
                    # c2 = f * c
                    v.wait_ge(as_sem, 4 * l + 1)
                    if l >= 1:
                        v.wait_ge(cs_sem, l)  # c_sb from step l-1 retired
                    v.tensor_mul(c2_sb[:, :], g_sb[:, 1024:2048], c_sb[:, :]).then_inc(
                        c2s_sem, 1
                    )
                    # t_sb reused as ig scratch: ig = i * g
                    v.wait_ge(as_sem, 4 * l + 2)
                    v.tensor_mul(t_sb[:, :], g_sb[:, 0:1024], g_sb[:, 2048:3072]).then_inc(
                        igs_sem, 1
                    )
                    # c = c2 + ig (same-engine RAW needs explicit sems)
                    v.wait_ge(c2s_sem, l + 1)
                    v.wait_ge(igs_sem, l + 1)
                    v.tensor_add(c_sb[:, :], c2_sb[:, :], t_sb[:, :]).then_inc(cs_sem, 1)
                    # h = o * tanh(c)
                    v.wait_ge(as_sem, 4 * l + 4)
                    if l >= 2 and not NO_DMA:
                        v.wait_ge(yw_sems[l % 2], 16 * (l // 2))
                    v.tensor_mul(
                        h_sb[:, (l % 2) * H : (l % 2 + 1) * H], g_sb[:, 3072:4096], t_sb[:, :]
                    ).then_inc(hs_sem, 1)
                    # hT evacs, 2 groups of 4 k-tiles
                    v.wait_ge(tp_sem, 2 * l + 1)
                    v.tensor_copy(hT_pack[:, 0:16], TA_ps[:, 0:16]).then_inc(es_sem, 1)
                    v.wait_ge(tp_sem, 2 * l + 2)
                    v.tensor_copy(hT_pack[:, 16:32], TB_ps[:, 0:16]).then_inc(es_sem, 1)

    nc.finalize()
    return nc


def _get_nc(w_dtype_str):
    if w_dtype_str not in _cached:
        _cached[w_dtype_str] = _build(w_dtype_str)
    return _cached[w_dtype_str]


W_DTYPE = os.environ.get("DLSTM_WDT", "f32r")


def kernel(x, W_ih, W_hh, b_ih, b_hh):
    from concourse.bass_utils import run_bass_kernel_spmd
    import ml_dtypes

    x = np.asarray(x, np.float32)
    W_ih = np.asarray(W_ih, np.float32)
    W_hh = np.asarray(W_hh, np.float32)
    bias = (np.asarray(b_ih, np.float32) + np.asarray(b_hh, np.float32))

    nc = _get_nc(W_DTYPE)

    # host-side prep
    wihT_ext = np.zeros((C + 128, G4), np.float32)
    wihT_ext[:C] = W_ih.T
    wihT_ext[C] = bias
    whhT = W_hh.T.copy()
    if W_DTYPE == "bf16":
        whhT = whhT.astype(ml_dtypes.bfloat16)
    ident = np.eye(4, dtype=np.float32)

    in_maps = []
    for c in range(NCORES):
        xT_ext = np.zeros((C + 128, T), np.float32)
        xT_ext[:C] = x[c].T
        xT_ext[C] = 1.0
        in_maps.append(
            {"xT": xT_ext, "wihT": wihT_ext, "whhT": whhT, "ident": ident}
        )

    res = run_bass_kernel_spmd(nc, in_maps, list(range(NCORES)))
    out = np.stack([res.results[c]["y"] for c in range(NCORES)], axis=0)
    return out.astype(np.float32)



# revision 22
# speedup vs baseline: 10.7075x; 10.7075x over previous
"""Dilated LSTM (B=8, T=2048, C=1024, H=1024, D=4) on 8 trn2 NeuronCores.

Strategy: data-parallel over batch (core c <- batch item c, its 4 dilation
chains). Everything is core-local (no cross-core traffic; remote-DMA and
collectives are not usable in this deployment).

Per core, a fused TRANSPOSED weight-stationary recurrence (no separate
input-GEMM phase, no DRAM staging of x@W_ih), software-pipelined as TWO
staggered chain groups (group 0 = dilation chains 0,1; group 1 = chains
2,3). The groups are independent recurrences, so their serial
h -> gates -> h dependency chains interleave on the engines half a step
apart, roughly halving the effective per-step latency.

  Per step l, group grp, gate chunk n (32 chunks of 128 gates), the PSUM
  accumulation group [128 gates, 2 chains] is:
      9 x-matmuls:  wih_tile[j,n] (stationary, [128,128] fp16)
                    @ xT[j-tile, 2 tokens] (moving, [128,2] fp16)
                    (9th j-tile is a host-appended ones-row carrying the
                     bias via a bias-row in wih)
    + 8 h-matmuls:  whh_tile[k,n] (stationary) @ hT[k-tile] (moving, [128,2])

  A matmul costs out_free_size (=2) cycles in the cost model, independent
  of K and M, so W_ih/W_hh never move through the PE as data: both are
  stationary operands and each matmul is ~1ns.

  PSUM: 8 zero regions (2KB each) = (2 groups x 2 parities) x
  {[i|f|o] region, [g] region}; the i/f/o region is separate from g so
  the sigmoid can start while g chunks still accumulate. All elementwise
  tail ops are [128, 16-32]. h is produced directly in the transposed hT
  layout the next step's matmuls consume (no transpose, no evacuation),
  written into a history ring (group-major columns) that doubles as the
  y output buffer (fp16), DMA'd to DRAM every S steps. x is streamed in
  transposed fp16 quarters.

Host reassembles y[b, t, h] from per-core yT slabs.
"""

import os
import sys

sys.path.insert(0, "/opt/trn_rl_repo")

import numpy as np

B, T, C, H, D = 8, 2048, 1024, 1024, 4
L = T // D          # 512 steps
G4 = 4 * H          # 4096 gates
KT = C // 128       # 8 k-tiles for h
KTX = KT + 1        # 9 k-tiles for x (incl bias ones-row block)
NCORES = 8
S = 8               # y writeback batch (steps per slab half)
HIST = 2 * S        # h history ring (cols of 32)
QT = 512            # x quarter tokens (128 steps)

N_STEPS = int(os.environ.get("DLSTM_STEPS", str(L)))  # dev override only
KCUT = int(os.environ.get("DLSTM_KCUT", str(KT)))  # timing experiment only

_cached = {}


def _cb(n):
    """PSUM col offset (2-wide chunks) within a group/parity region pair.

    Region layout per (group, parity): 2KB region 0 = [i(0:16)|f(16:32)|
    o(32:48)], 2KB region 1 (base+512) = g(0:16).
    """
    if n < 8:
        return n * 2                # i  (region 0)
    if n < 16:
        return 16 + (n - 8) * 2     # f  (region 0)
    if n < 24:
        return 512 + (n - 16) * 2   # g  (region 1)
    return 32 + (n - 24) * 2        # o  (region 0)


def _build():
    import concourse.bass as bass  # noqa: F401
    import concourse.bacc as bacc
    import concourse.mybir as mybir

    F32 = mybir.dt.float32
    F16 = mybir.dt.float16
    SIG = mybir.ActivationFunctionType.Sigmoid
    TANH = mybir.ActivationFunctionType.Tanh

    nc = bacc.Bacc(None, target_bir_lowering=False)

    xT = nc.dram_tensor("xT", [C + 128, T], F16, kind="ExternalInput")
    wih = nc.dram_tensor("wih", [C + 128, G4], F16, kind="ExternalInput")
    whh = nc.dram_tensor("whh", [C, G4], F16, kind="ExternalInput")
    yT = nc.dram_tensor("yT", [128, L * 32], F16, kind="ExternalOutput")

    LS = N_STEPS
    NQ = max(1, (LS * 4 + QT - 1) // QT)  # quarters actually consumed

    from contextlib import ExitStack

    with ExitStack() as es:
        sems = {}
        names = ["wl", "xtq0", "xtq1", "xqf", "ms", "ys0", "ys1"]
        for g in range(2):
            for nm in ("pifo", "po", "pg", "sg", "so", "tg", "m1", "m2", "cr",
                       "tc", "hs"):
                names.append(f"{nm}{g}")
        for nm in names:
            sems[nm] = es.enter_context(nc.semaphore(nm + "_sem"))
        wl, xqf, ms = sems["wl"], sems["xqf"], sems["ms"]
        xtq = (sems["xtq0"], sems["xtq1"])
        ys = (sems["ys0"], sems["ys1"])
        pifo = (sems["pifo0"], sems["pifo1"])
        po = (sems["po0"], sems["po1"])
        pg = (sems["pg0"], sems["pg1"])
        sg = (sems["sg0"], sems["sg1"])
        so = (sems["so0"], sems["so1"])
        tg = (sems["tg0"], sems["tg1"])
        m1 = (sems["m10"], sems["m11"])
        m2 = (sems["m20"], sems["m21"])
        cr = (sems["cr0"], sems["cr1"])
        tc = (sems["tc0"], sems["tc1"])
        hs = (sems["hs0"], sems["hs1"])

        # ---- SBUF (per-partition: 72 + 64 + 18 + ~2 KB) ----
        wih_sb = es.enter_context(nc.sbuf_tensor("wih_sb", [128, KTX * G4], F16))
        whh_sb = es.enter_context(nc.sbuf_tensor("whh_sb", [128, KT * G4], F16))
        xq_sb = es.enter_context(nc.sbuf_tensor("xq_sb", [128, 2 * KTX * QT], F16))
        # hist col layout per step: grp*16 + k*2 + chain_in_group
        hist = es.enter_context(nc.sbuf_tensor("hist", [128, HIST * 32], F16))
        # per group: [g_tanh(0:16) | c(16:32)] at cols grp*32
        gc_sb = es.enter_context(nc.sbuf_tensor("gc_sb", [128, 64], F32))
        # per group: [ig(0:16) | fc(16:32)] at cols grp*32
        p_sb = es.enter_context(nc.sbuf_tensor("p_sb", [128, 64], F32))
        tc_sb = es.enter_context(nc.sbuf_tensor("tc_sb", [128, 32], F32))
        # PSUM: all 8 zero regions; (grp, parity) pair base = (grp*2+qp)*1024
        gps = es.enter_context(nc.psum_tensor("gps", [128, 8 * 512], F32))

        blk = es.enter_context(nc.Block())

        def rb(grp, qp):
            return (grp * 2 + qp) * 1024

        # ---------------- sync engine: loads + y writeback ----------
        @blk.sync
        def _(s):
            for j in range(KTX):
                s.dma_start(
                    wih_sb[:, j * G4: (j + 1) * G4], wih[j * 128: (j + 1) * 128, :]
                ).then_inc(wl, 16)
            for k in range(KT):
                s.dma_start(
                    whh_sb[:, k * G4: (k + 1) * G4], whh[k * 128: (k + 1) * 128, :]
                ).then_inc(wl, 16)

            def _xq_load(s, q):
                if q >= 2:
                    s.wait_ge(xqf, q - 1)  # quarter q-2 fully consumed
                base = (q % 2) * KTX * QT
                for j in range(KTX):
                    s.dma_start(
                        xq_sb[:, base + j * QT: base + (j + 1) * QT],
                        xT[j * 128: (j + 1) * 128, q * QT: (q + 1) * QT],
                    ).then_inc(xtq[q % 2], 16)

            _xq_load(s, 0)
            if NQ > 1:
                _xq_load(s, 1)
            for l in range(LS):
                # quarter q+2 reload: emitted at l%128==120 so the xqf wait
                # (satisfied once the PE finishes step q*128+127) sits AFTER
                # the y-slab ships the DVE needs to reach that step -- at
                # l%128==0 it would deadlock (SP blocked on PE, PE on DVE,
                # DVE on a y ship queued behind the SP wait)
                if l % 128 == 120 and l // 128 + 2 < NQ:
                    _xq_load(s, l // 128 + 2)
                if l % S == 3 and l >= S:
                    b = l // S - 1
                    s.wait_ge(hs[0], (b + 1) * S + 1)
                    s.wait_ge(hs[1], (b + 1) * S + 1)
                    s.dma_start(
                        yT[:, b * S * 32: (b + 1) * S * 32],
                        hist[:, (b % 2) * S * 32: (b % 2 + 1) * S * 32],
                    ).then_inc(ys[b % 2], 16)
            b = LS // S - 1
            if b >= 0:
                s.wait_ge(hs[0], (b + 1) * S + 1)
                s.wait_ge(hs[1], (b + 1) * S + 1)
                s.dma_start(
                    yT[:, b * S * 32: (b + 1) * S * 32],
                    hist[:, (b % 2) * S * 32: (b % 2 + 1) * S * 32],
                ).then_inc(ys[b % 2], 16)
            nb = LS // S
            s.wait_ge(ys[0], 16 * ((nb + 1) // 2))
            s.wait_ge(ys[1], 16 * (nb // 2))

        # ---------------- gpsimd: state init ----------------
        @blk.gpsimd
        def _(g):
            # h(-1) = 0 for both groups; c = 0 per group; each memset carries
            # one init sem (one update per instruction)
            g.memset(hist[:, (HIST - 1) * 32: HIST * 32], 0.0).then_inc(hs[0], 1)
            g.memset(gc_sb[:, 16:32], 0.0).then_inc(hs[1], 1)
            g.memset(gc_sb[:, 48:64], 0.0).then_inc(ms, 1)

        # ---------------- tensor engine ----------------
        @blk.tensor
        def _(t):
            t.wait_ge(wl, 16 * (KTX + KT))
            t.wait_ge(xtq[0], 16 * KTX)
            # chunk order: i,f,o first (sigmoid can start while g chunks
            # still accumulate in their own zero region), then g
            order = list(range(0, 16)) + list(range(24, 32)) + list(range(16, 24))
            for l in range(LS):
                qp = l % 2
                q = (4 * l) // QT
                if l % 128 == 0 and q > 0:
                    t.wait_ge(xtq[q % 2], 16 * KTX * (q // 2 + 1))
                hcol = ((l - 1) % HIST) * 32
                xoff = (q % 2) * KTX * QT + (l % 128) * 4
                for grp in range(2):
                    base = rb(grp, qp)
                    t.wait_ge(hs[grp], l + 1)
                    for ci, n in enumerate(order):
                        for j in range(KTX):
                            xm = t.matmul(
                                gps[:, base + _cb(n): base + _cb(n) + 2],
                                wih_sb[:, j * G4 + n * 128: j * G4
                                       + (n + 1) * 128],
                                xq_sb[:, xoff + j * QT + grp * 2: xoff + j * QT
                                      + grp * 2 + 2],
                                start=(j == 0),
                                stop=False,
                            )
                            if (ci == 31 and j == KTX - 1 and grp == 1
                                    and l % 128 == 127):
                                xm.then_inc(xqf, 1)  # quarter consumed
                        for k in range(KCUT):
                            mm = t.matmul(
                                gps[:, base + _cb(n): base + _cb(n) + 2],
                                whh_sb[:, k * G4 + n * 128: k * G4
                                       + (n + 1) * 128],
                                hist[:, hcol + grp * 16 + k * 2: hcol
                                     + grp * 16 + (k + 1) * 2],
                                start=False,
                                stop=(k == KCUT - 1),
                            )
                        if ci == 23:
                            mm.then_inc(pifo[grp], 1)
                        elif ci == 31:
                            mm.then_inc(pg[grp], 1)

        # ---------------- scalar (ACT) ----------------
        @blk.scalar
        def _(a):
            for l in range(LS):
                qp = l % 2
                for grp in range(2):
                    base = rb(grp, qp)
                    a.wait_ge(pifo[grp], l + 1)
                    a.activation(
                        gps[:, base: base + 48], gps[:, base: base + 48], SIG
                    ).then_inc(sg[grp], 1)
                    a.wait_ge(pg[grp], l + 1)
                    a.activation(
                        gc_sb[:, grp * 32: grp * 32 + 16],
                        gps[:, base + 512: base + 528], TANH
                    ).then_inc(tg[grp], 1)
                for grp in range(2):
                    a.wait_ge(cr[grp], l + 1)
                    a.activation(
                        tc_sb[:, grp * 16: (grp + 1) * 16],
                        gc_sb[:, grp * 32 + 16: grp * 32 + 32], TANH
                    ).then_inc(tc[grp], 1)

        # ---------------- vector (DVE) ----------------
        @blk.vector
        def _(v):
            for l in range(LS):
                qp = l % 2
                for grp in range(2):
                    base = rb(grp, qp)
                    v.wait_ge(sg[grp], l + 1)
                    v.wait_ge(tg[grp], l + 1)
                    if l == 0:
                        v.wait_ge(ms, 1)
                    else:
                        v.wait_ge(cr[grp], l)  # c RAW + p_sb WAR vs step l-1
                    # [ig | fc] = [i,f](psum) * [g_tanh, c](sbuf)
                    v.tensor_mul(
                        p_sb[:, grp * 32: (grp + 1) * 32],
                        gps[:, base: base + 32],
                        gc_sb[:, grp * 32: (grp + 1) * 32],
                    ).then_inc(m1[grp], 1)
                    v.wait_ge(m1[grp], l + 1)
                    v.tensor_add(
                        gc_sb[:, grp * 32 + 16: grp * 32 + 32],
                        p_sb[:, grp * 32: grp * 32 + 16],
                        p_sb[:, grp * 32 + 16: grp * 32 + 32],
                    ).then_inc(cr[grp], 1)
                for grp in range(2):
                    base = rb(grp, qp)
                    if grp == 0 and l % S == 0 and l >= 2 * S:
                        b_e = l // S
                        v.wait_ge(ys[b_e % 2], 16 * (b_e // 2))
                    v.wait_ge(tc[grp], l + 1)
                    v.tensor_mul(
                        hist[:, (l % HIST) * 32 + grp * 16: (l % HIST) * 32
                             + (grp + 1) * 16],
                        gps[:, base + 32: base + 48],
                        tc_sb[:, grp * 16: (grp + 1) * 16],
                    ).then_inc(hs[grp], 1)

    nc.finalize()
    return nc


def _get_nc():
    if "nc" not in _cached:
        _cached["nc"] = _build()
    return _cached["nc"]


def kernel(x, W_ih, W_hh, b_ih, b_hh):
    from concourse.bass_utils import run_bass_kernel_spmd

    x = np.asarray(x, np.float32)
    W_ih = np.asarray(W_ih, np.float32)
    W_hh = np.asarray(W_hh, np.float32)
    bias = np.asarray(b_ih, np.float32) + np.asarray(b_hh, np.float32)

    nc = _get_nc()

    wih_ext = np.zeros((C + 128, G4), np.float16)
    wih_ext[:C] = W_ih.T.astype(np.float16)
    wih_ext[C] = bias.astype(np.float16)
    whh16 = W_hh.T.astype(np.float16)

    in_maps = []
    for c in range(NCORES):
        xT_ext = np.zeros((C + 128, T), np.float16)
        xT_ext[:C] = x[c].T.astype(np.float16)
        xT_ext[C] = 1.0
        in_maps.append({"xT": xT_ext, "wih": wih_ext, "whh": whh16})

    res = run_bass_kernel_spmd(nc, in_maps, list(range(NCORES)))
    out = np.zeros((B, T, H), np.float32)
    for c in range(NCORES):
        yt = np.asarray(res.results[c]["yT"], np.float16)  # [128, L*32]
        # col = l*32 + grp*16 + k*2 + cc ; chain d = grp*2 + cc
        arr = yt.reshape(128, L, 2, KT, 2).astype(np.float32)
        # y[4l + 2*grp + cc, 128k+p] = arr[p, l, grp, k, cc]
        out[c] = arr.transpose(1, 2, 4, 3, 0).reshape(T, H)
    return out
